# revision 6
# baseline (speedup 1.0000x reference)
"""Trainium2 Bass kernel for a 3-layer conditional LSTM (SMILES RNN) with
encoder/decoder feedback.

Math reformulation (verified vs the jax reference):
  - The decoder->encoder feedback path is folded through the rank-47 logits:
      gates0 = A0 @ logits_prev + Wp0 @ props + Whh0 @ h0 + b0c
    with A0 = w_ih0[:, :H] @ enc_w, Wp0 = w_ih0[:, H:], and
    b0c = w_ih0[:, :H] @ enc_b + b_ih0 + b_hh0.  [A0 | Wp0 | b0c] forms one
    K=52 augmented contraction whose stationary operand is
    [logits.T; props.T; ones].
  - t=0 is uniform with logits_init = onehot(1) (the start token encodes to
    exactly enc_w @ onehot1 + enc_b).
  - Logits are produced per-step into an SBUF history buffer and DMA'd out
    once at the end.

Distribution: pure data parallel, batch 128 -> 16 rows per core, weights
replicated; the sequential scan stays core-local (no collectives).

Layout: activations batch-on-partition [16, *]; weights are the *moving*
matmul operand streamed as float32r (full fp32 storage, ~1e-4 matmul
accuracy, 1 cycle/row on TRN2 for moving dim >= 256).  The per-step h must
be transposed ([16,512] -> 4x [128,16]) to serve as the next stationary
operand; done on the PE with an identity matmul.
"""

import numpy as np

B, T, H, O, P, NL = 128, 64, 512, 47, 4, 3
G = 4 * H
NCORES = 8
BL = B // NCORES
KAUG = O + P + 1  # 52
OP = 48  # O padded to even width (fp32r ISA: innermost free count must be even)


def _build_nc(t_steps):
    import concourse.mybir as mybir
    import concourse.tile as tile
    from concourse import bacc
    from concourse.masks import make_identity

    F32 = mybir.dt.float32
    F32R = mybir.dt.float32r
    ACT = mybir.ActivationFunctionType

    nc = bacc.Bacc(None, target_bir_lowering=False)

    w0aug_d = nc.dram_tensor("w0aug", [KAUG, G], F32R, kind="ExternalInput")
    whh0_d = nc.dram_tensor("whh0", [128, 4, G], F32R, kind="ExternalInput")
    w1_d = nc.dram_tensor("w1", [128, 8, G], F32R, kind="ExternalInput")
    w2_d = nc.dram_tensor("w2", [128, 8, G], F32R, kind="ExternalInput")
    dec_d = nc.dram_tensor("dec", [128, 4, OP], F32R, kind="ExternalInput")
    b1_d = nc.dram_tensor("b1", [1, G], F32R, kind="ExternalInput")
    b2_d = nc.dram_tensor("b2", [1, G], F32R, kind="ExternalInput")
    decb_d = nc.dram_tensor("dec_b", [1, OP], F32R, kind="ExternalInput")
    xaug_d = nc.dram_tensor("xaug0", [KAUG, BL], F32R, kind="ExternalInput")
    init_d = nc.dram_tensor("init", [128, NL * 4 * BL + BL], F32R, kind="ExternalInput")
    out_d = nc.dram_tensor("out", [BL, t_steps * O], F32, kind="ExternalOutput")

    with tile.TileContext(nc) as tc:
        with (
            tc.tile_pool(name="weights", bufs=1) as wp,
            tc.tile_pool(name="state", bufs=1) as sp,
            tc.tile_pool(name="htmp", bufs=1) as hp,
            tc.tile_pool(name="gpool", bufs=1, space="PSUM") as gp,
            tc.tile_pool(name="tpool", bufs=3, space="PSUM") as tp,
            tc.tile_pool(name="lpool", bufs=1, space="PSUM") as lp,
        ):
            w0aug = wp.tile([KAUG, G], F32R)
            nc.gpsimd.dma_start(w0aug[:], w0aug_d[:])
            whh0 = wp.tile([128, 4, G], F32R)
            nc.gpsimd.dma_start(whh0[:], whh0_d[:])
            w1 = wp.tile([128, 8, G], F32R)
            nc.gpsimd.dma_start(w1[:], w1_d[:])
            w2 = wp.tile([128, 8, G], F32R)
            nc.gpsimd.dma_start(w2[:], w2_d[:])
            dec = wp.tile([128, 4, OP], F32R)
            nc.gpsimd.dma_start(dec[:], dec_d[:])
            b1 = wp.tile([1, G], F32R)
            nc.gpsimd.dma_start(b1[:], b1_d[:])
            b2 = wp.tile([1, G], F32R)
            nc.gpsimd.dma_start(b2[:], b2_d[:])
            dec_b = wp.tile([1, OP], F32R)
            nc.gpsimd.dma_start(dec_b[:], decb_d[:])

            xaug = sp.tile([KAUG, BL], F32R)
            nc.gpsimd.dma_start(xaug[:], xaug_d[:])
            initt = sp.tile([128, NL * 4 * BL + BL], F32R)
            nc.gpsimd.dma_start(initt[:], init_d[:])
            hT = initt[:, :NL * 4 * BL]
            ones_t = initt[0:1, NL * 4 * BL:NL * 4 * BL + BL]
            ident = sp.tile([BL, BL], F32)
            make_identity(nc, ident)
            cs = []
            for l in range(NL):
                c = sp.tile([BL, H], F32, tag=f"c{l}")
                nc.vector.memset(c[:], 0.0)
                cs.append(c)

            def r(ap):
                return ap

            def hT_sl(l, k):
                j = (l * 4 + k) * BL
                return initt[:, j:j + BL]

            def lstm_pointwise(gps, c):
                # ACT moves gates PSUM->SBUF with the nonlinearity fused;
                # DVE ops then stay SBUF-only (HW: max one PSUM read per op).
                ga = hp.tile([BL, G], F32, tag="gact")
                i_ = ga[:, 0 * H:1 * H]
                f_ = ga[:, 1 * H:2 * H]
                g_ = ga[:, 2 * H:3 * H]
                o_ = ga[:, 3 * H:4 * H]
                nc.scalar.activation(i_, gps[:, 0 * H:1 * H], ACT.Sigmoid)
                nc.scalar.activation(f_, gps[:, 1 * H:2 * H], ACT.Sigmoid)
                nc.scalar.activation(g_, gps[:, 2 * H:3 * H], ACT.Tanh)
                nc.scalar.activation(o_, gps[:, 3 * H:4 * H], ACT.Sigmoid)
                nc.vector.tensor_mul(i_, i_, g_)   # sig(i)*tanh(g)
                nc.vector.tensor_mul(f_, f_, c)    # sig(f)*c
                nc.vector.tensor_add(c, i_, f_)    # new c
                nc.scalar.activation(g_, c, ACT.Tanh)  # tanh(c), scratch in g slot
                h = ga[:, 0 * H:1 * H]             # reuse i slot for h
                nc.vector.tensor_mul(h, o_, g_)
                return h

            def transposes(h, l):
                for k in range(4):
                    tps = tp.tile([128, BL], F32, tag="tps")
                    nc.tensor.transpose(tps[:], h[:, k * 128:(k + 1) * 128], ident[:])
                    nc.vector.tensor_copy(hT_sl(l, k), tps[:])

            for t in range(t_steps):
                # ---- cell 0: gates = Whh0@h0 + [A0|Wp0|b0c]@xaug
                gps = gp.tile([BL, G], F32, tag="gates")
                for n in range(4):
                    nsl = slice(n * H, (n + 1) * H)
                    for k in range(4):
                        nc.tensor.matmul(gps[:, nsl], r(hT_sl(0, k)), r(whh0[:, k, nsl]),
                                         start=(k == 0), stop=False)
                    nc.tensor.matmul(gps[:, nsl], r(xaug[:]), r(w0aug[:, nsl]),
                                     start=False, stop=True)
                h = lstm_pointwise(gps, cs[0])
                transposes(h, 0)

                # ---- cells 1, 2: gates = bias + Whh@h_own + Wih@h_below
                for l, wl, bl_t in ((1, w1, b1), (2, w2, b2)):
                    gps = gp.tile([BL, G], F32, tag="gates")
                    for n in range(4):
                        nsl = slice(n * H, (n + 1) * H)
                        nc.tensor.matmul(gps[:, nsl], r(ones_t), r(bl_t[:, nsl]),
                                         start=True, stop=False)
                        for k in range(4):
                            nc.tensor.matmul(gps[:, nsl], r(hT_sl(l, k)), r(wl[:, k, nsl]),
                                             start=False, stop=False)
                        for k in range(4):
                            nc.tensor.matmul(gps[:, nsl], r(hT_sl(l - 1, k)), r(wl[:, 4 + k, nsl]),
                                             start=False, stop=(k == 3))
                    h = lstm_pointwise(gps, cs[l])
                    transposes(h, l)

                # ---- logits = dec_b + dec @ h2
                lps = lp.tile([BL, OP], F32, tag="lps")
                nc.tensor.matmul(lps[:], r(ones_t), r(dec_b[:]), start=True, stop=False)
                for k in range(4):
                    nc.tensor.matmul(lps[:], r(hT_sl(2, k)), r(dec[:, k, :]),
                                     start=False, stop=(k == 3))
                lt = hp.tile([BL, O], F32, tag="lt")
                nc.scalar.activation(lt[:], lps[:, :O], ACT.Copy)
                nc.sync.dma_start(out_d[:, t * O:(t + 1) * O], lt[:])
                # logits.T -> xaug rows 0:O for the next step
                tps = tp.tile([128, BL], F32, tag="tps")
                nc.tensor.transpose(tps[:O, :], lt[:], ident[:])
                nc.vector.tensor_copy(xaug[0:O, :], tps[:O, :])

    nc.compile()
    return nc


def _init_const():
    init = np.zeros((128, NL * 4 * BL + BL), np.float32)
    init[0, NL * 4 * BL:] = 1.0
    return init


def _host_fold(inputs):
    """Fold encoder/decoder/properties/biases into per-core device inputs."""
    ins = {k: np.asarray(v) for k, v in inputs.items()}
    w_ih0 = ins["w_ih0"].astype(np.float32)
    w_hh0 = ins["w_hh0"].astype(np.float32)
    enc_w = ins["enc_w"].astype(np.float32)
    enc_b = ins["enc_b"].astype(np.float32)
    dec_w = ins["dec_w"].astype(np.float32)
    dec_b = ins["dec_b"].astype(np.float32)
    prop = ins["properties"].astype(np.float32)

    Wx0 = w_ih0[:, :H]
    Wp0 = w_ih0[:, H:]
    A0 = Wx0 @ enc_w                                   # [G, O]
    b0c = Wx0 @ enc_b + ins["b_ih0"] + ins["b_hh0"]    # [G]
    w0aug = np.ascontiguousarray(
        np.concatenate([A0.T, Wp0.T, b0c[None, :].astype(np.float32)], axis=0),
        dtype=np.float32)                              # [52, G]

    def chunked(wT, nk):  # [nk*128, G] -> [128, nk, G]
        return np.ascontiguousarray(
            wT.reshape(nk, 128, wT.shape[1]).transpose(1, 0, 2), dtype=np.float32)

    whh0 = chunked(w_hh0.T, 4)
    W1cat = np.concatenate([ins["w_hh_rest"][0].T, ins["w_ih_rest"][0].T], axis=0)
    W2cat = np.concatenate([ins["w_hh_rest"][1].T, ins["w_ih_rest"][1].T], axis=0)
    w1 = chunked(W1cat.astype(np.float32), 8)
    w2 = chunked(W2cat.astype(np.float32), 8)
    decT_pad = np.zeros((H, OP), np.float32)
    decT_pad[:, :O] = dec_w.T
    dec = chunked(decT_pad, 4)                         # [128, 4, OP]
    b1 = (ins["b_ih_rest"][0] + ins["b_hh_rest"][0]).astype(np.float32)[None, :]
    b2 = (ins["b_ih_rest"][1] + ins["b_hh_rest"][1]).astype(np.float32)[None, :]

    shared = {
        "w0aug": w0aug, "whh0": whh0, "w1": w1, "w2": w2, "dec": dec,
        "b1": np.ascontiguousarray(b1), "b2": np.ascontiguousarray(b2),
        "dec_b": np.ascontiguousarray(
            np.concatenate([dec_b, np.zeros(OP - O, np.float32)])[None, :], dtype=np.float32),
        "init": _init_const(),
    }
    in_maps = []
    for cid in range(NCORES):
        xaug = np.zeros((KAUG, BL), np.float32)
        xaug[1, :] = 1.0                               # logits_init = onehot(1)
        xaug[O:O + P, :] = prop[cid * BL:(cid + 1) * BL, :].T
        xaug[O + P, :] = 1.0
        in_maps.append({**shared, "xaug0": np.ascontiguousarray(xaug)})
    return in_maps


_NC_CACHE = {}


def _run(inputs, t_steps):
    from concourse.bass_utils import run_bass_kernel_spmd

    if t_steps not in _NC_CACHE:
        _NC_CACHE[t_steps] = _build_nc(t_steps)
    nc = _NC_CACHE[t_steps]
    in_maps = _host_fold(inputs)
    res = run_bass_kernel_spmd(nc, in_maps, core_ids=list(range(NCORES)))
    outs = [res.results[cid]["out"].reshape(BL, t_steps, O) for cid in range(NCORES)]
    return np.concatenate(outs, axis=0)


def kernel(**inputs):
    t_steps = np.asarray(inputs["x"]).shape[1]
    return _run(inputs, t_steps)


# revision 8
# speedup vs baseline: 2218.5180x; 2218.5180x over previous
"""Trainium2 Bass kernel for a 3-layer conditional LSTM (SMILES RNN) with
encoder/decoder feedback.

Math reformulation (verified vs the jax reference):
  - The decoder->encoder feedback path is folded through the rank-47 logits:
      gates0 = A0 @ logits_prev + Wp0 @ props + Whh0 @ h0 + b0c
    with A0 = w_ih0[:, :H] @ enc_w, Wp0 = w_ih0[:, H:], and
    b0c = w_ih0[:, :H] @ enc_b + b_ih0 + b_hh0.  [A0 | Wp0 | b0c] forms one
    K=52 augmented contraction whose stationary operand is
    [logits.T; props.T; ones].
  - t=0 is uniform with logits_init = onehot(1) (the start token encodes to
    exactly enc_w @ onehot1 + enc_b).
  - Logits are produced per-step into an SBUF history buffer and DMA'd out
    once at the end.

Distribution: pure data parallel, batch 128 -> 16 rows per core, weights
replicated; the sequential scan stays core-local (no collectives).

Layout: activations batch-on-partition [16, *]; weights are the *moving*
matmul operand streamed as float32r (full fp32 storage, ~1e-4 matmul
accuracy, 1 cycle/row on TRN2 for moving dim >= 256).  The per-step h must
be transposed ([16,512] -> 4x [128,16]) to serve as the next stationary
operand; done on the PE with an identity matmul.
"""

import numpy as np

B, T, H, O, P, NL = 128, 64, 512, 47, 4, 3
G = 4 * H
NCORES = 8
BL = B // NCORES
KAUG = O + P + 1  # 52
OP = 48  # O padded to even width (fp32r ISA: innermost free count must be even)


def _build_nc(t_steps):
    import concourse.mybir as mybir
    import concourse.tile as tile
    from concourse import bacc
    from concourse.masks import make_identity

    F32 = mybir.dt.float32
    F32R = mybir.dt.float32r
    ACT = mybir.ActivationFunctionType

    nc = bacc.Bacc(None, target_bir_lowering=False)

    w0aug_d = nc.dram_tensor("w0aug", [KAUG, G], F32R, kind="ExternalInput")
    whh0_d = nc.dram_tensor("whh0", [128, 4, G], F32R, kind="ExternalInput")
    w1_d = nc.dram_tensor("w1", [128, 8, G], F32R, kind="ExternalInput")
    w2_d = nc.dram_tensor("w2", [128, 8, G], F32R, kind="ExternalInput")
    dec_d = nc.dram_tensor("dec", [128, 4, OP], F32R, kind="ExternalInput")
    b1_d = nc.dram_tensor("b1", [1, G], F32R, kind="ExternalInput")
    b2_d = nc.dram_tensor("b2", [1, G], F32R, kind="ExternalInput")
    decb_d = nc.dram_tensor("dec_b", [1, OP], F32R, kind="ExternalInput")
    xaug_d = nc.dram_tensor("xaug0", [KAUG, BL], F32R, kind="ExternalInput")
    init_d = nc.dram_tensor("init", [128, NL * 4 * BL + BL], F32R, kind="ExternalInput")
    out_d = nc.dram_tensor("out", [BL, t_steps * O], F32, kind="ExternalOutput")

    with tile.TileContext(nc) as tc:
        with (
            tc.tile_pool(name="weights", bufs=1) as wp,
            tc.tile_pool(name="state", bufs=1) as sp,
            tc.tile_pool(name="htmp", bufs=1) as hp,
            tc.tile_pool(name="gpool", bufs=6, space="PSUM") as gp,
            tc.tile_pool(name="tpool", bufs=2, space="PSUM") as tp,
        ):
            w0aug = wp.tile([KAUG, G], F32R)
            nc.gpsimd.dma_start(w0aug[:], w0aug_d[:])
            whh0 = wp.tile([128, 4, G], F32R)
            nc.gpsimd.dma_start(whh0[:], whh0_d[:])
            w1 = wp.tile([128, 8, G], F32R)
            nc.gpsimd.dma_start(w1[:], w1_d[:])
            w2 = wp.tile([128, 8, G], F32R)
            nc.gpsimd.dma_start(w2[:], w2_d[:])
            dec = wp.tile([128, 4, OP], F32R)
            nc.gpsimd.dma_start(dec[:], dec_d[:])
            b1 = wp.tile([1, G], F32R)
            nc.gpsimd.dma_start(b1[:], b1_d[:])
            b2 = wp.tile([1, G], F32R)
            nc.gpsimd.dma_start(b2[:], b2_d[:])
            dec_b = wp.tile([1, OP], F32R)
            nc.gpsimd.dma_start(dec_b[:], decb_d[:])

            xaug = sp.tile([KAUG, BL], F32R)
            nc.gpsimd.dma_start(xaug[:], xaug_d[:])
            initt = sp.tile([128, NL * 4 * BL + BL], F32R)
            nc.gpsimd.dma_start(initt[:], init_d[:])
            hT = initt[:, :NL * 4 * BL]
            ones_t = initt[0:1, NL * 4 * BL:NL * 4 * BL + BL]
            ident = sp.tile([BL, BL], F32)
            make_identity(nc, ident)
            cs = []
            for l in range(NL):
                c = sp.tile([BL, H], F32, tag=f"c{l}")
                nc.vector.memset(c[:], 0.0)
                cs.append(c)

            def r(ap):
                return ap

            def hT_sl(l, k):
                j = (l * 4 + k) * BL
                return initt[:, j:j + BL]

            def transposes(h, l):
                for k in range(4):
                    tps = tp.tile([128, BL], F32, tag="tps")
                    nc.tensor.transpose(tps[:], h[:, k * 128:(k + 1) * 128], ident[:])
                    nc.vector.tensor_copy(hT_sl(l, k), tps[:])

            def emit_hh0(t):
                """cell0 hh matmuls for step t into fresh per-gate psum chunks."""
                # (name= explicit: list-comp allocation defeats name inference)
                chunks = [gp.tile([BL, H], F32, tag="g", name=f"g0_{t}_{n}") for n in range(4)]
                for n in range(4):
                    nsl = slice(n * H, (n + 1) * H)
                    for k in range(4):
                        nc.tensor.matmul(chunks[n][:], r(hT_sl(0, k)), r(whh0[:, k, nsl]),
                                         start=(k == 0), stop=False)
                return chunks

            def lstm_pointwise(chunks, c):
                ga = hp.tile([BL, G], F32, tag="gact")
                i_ = ga[:, 0 * H:1 * H]
                f_ = ga[:, 1 * H:2 * H]
                g_ = ga[:, 2 * H:3 * H]
                o_ = ga[:, 3 * H:4 * H]
                nc.scalar.activation(i_, chunks[0][:], ACT.Sigmoid)
                nc.scalar.activation(f_, chunks[1][:], ACT.Sigmoid)
                nc.scalar.activation(g_, chunks[2][:], ACT.Tanh)
                nc.vector.tensor_mul(i_, i_, g_)   # sig(i)*tanh(g)
                nc.vector.tensor_mul(f_, f_, c)    # sig(f)*c
                nc.scalar.activation(o_, chunks[3][:], ACT.Sigmoid)
                nc.vector.tensor_add(c, i_, f_)    # new c
                nc.scalar.activation(g_, c, ACT.Tanh)  # tanh(c), scratch in g slot
                h = ga[:, 0 * H:1 * H]             # reuse i slot for h
                nc.vector.tensor_mul(h, o_, g_)
                return h

            # prologue: cell0 hh matmuls for t=0
            g0_chunks = emit_hh0(0)
            for t in range(t_steps):
                # (1) cell1 independent: bias + own-h  [dep: hT1(t-1)]
                g1_chunks = [gp.tile([BL, H], F32, tag="g", name=f"g0_{t}_{n}") for n in range(4)]
                for n in range(4):
                    nsl = slice(n * H, (n + 1) * H)
                    nc.tensor.matmul(g1_chunks[n][:], r(ones_t), r(b1[:, nsl]),
                                     start=True, stop=False)
                    for k in range(4):
                        nc.tensor.matmul(g1_chunks[n][:], r(hT_sl(1, k)), r(w1[:, k, nsl]),
                                         start=False, stop=False)
                # (2) cell0 aug matmuls  [dep: xaug(t-1 tail)]
                for n in range(4):
                    nsl = slice(n * H, (n + 1) * H)
                    nc.tensor.matmul(g0_chunks[n][:], r(xaug[:]), r(w0aug[:, nsl]),
                                     start=False, stop=True)
                # (3) cell0 pointwise
                h0 = lstm_pointwise(g0_chunks, cs[0])
                # (4) cell2 independent: bias + own-h  [dep: hT2(t-1)] — fills pointwise0
                g2_chunks = [gp.tile([BL, H], F32, tag="g", name=f"g0_{t}_{n}") for n in range(4)]
                for n in range(4):
                    nsl = slice(n * H, (n + 1) * H)
                    nc.tensor.matmul(g2_chunks[n][:], r(ones_t), r(b2[:, nsl]),
                                     start=True, stop=False)
                    for k in range(4):
                        nc.tensor.matmul(g2_chunks[n][:], r(hT_sl(2, k)), r(w2[:, k, nsl]),
                                         start=False, stop=False)
                # (5) h0 -> hT0
                transposes(h0, 0)
                # (6) cell1 input matmuls  [dep: hT0(t)]
                for n in range(4):
                    nsl = slice(n * H, (n + 1) * H)
                    for k in range(4):
                        nc.tensor.matmul(g1_chunks[n][:], r(hT_sl(0, k)), r(w1[:, 4 + k, nsl]),
                                         start=False, stop=(k == 3))
                # (7) cell1 pointwise
                h1 = lstm_pointwise(g1_chunks, cs[1])
                # (9) h1 -> hT1
                transposes(h1, 1)
                # (10) cell2 input matmuls  [dep: hT1(t)]
                for n in range(4):
                    nsl = slice(n * H, (n + 1) * H)
                    for k in range(4):
                        nc.tensor.matmul(g2_chunks[n][:], r(hT_sl(1, k)), r(w2[:, 4 + k, nsl]),
                                         start=False, stop=(k == 3))
                # (11) cell2 pointwise
                h2 = lstm_pointwise(g2_chunks, cs[2])
                # (12) next step's cell0 hh — fills pointwise2  [dep: hT0(t)]
                if t + 1 < t_steps:
                    g0_chunks = emit_hh0(t + 1)
                # (13) h2 -> hT2
                transposes(h2, 2)
                # (14) logits = dec_b + dec @ h2
                lps = tp.tile([BL, OP], F32, tag="tps")
                nc.tensor.matmul(lps[:], r(ones_t), r(dec_b[:]), start=True, stop=False)
                for k in range(4):
                    nc.tensor.matmul(lps[:], r(hT_sl(2, k)), r(dec[:, k, :]),
                                     start=False, stop=(k == 3))
                # (15) logits tail: out DMA + xaug update
                lt = hp.tile([BL, O], F32, tag="lt")
                nc.vector.tensor_copy(lt[:], lps[:, :O])
                nc.sync.dma_start(out_d[:, t * O:(t + 1) * O], lt[:])
                tps = tp.tile([128, BL], F32, tag="tps")
                nc.tensor.transpose(tps[:O, :], lt[:], ident[:])
                nc.vector.tensor_copy(xaug[0:O, :], tps[:O, :])

    nc.compile()
    return nc


def _init_const():
    init = np.zeros((128, NL * 4 * BL + BL), np.float32)
    init[0, NL * 4 * BL:] = 1.0
    return init


def _host_fold(inputs):
    """Fold encoder/decoder/properties/biases into per-core device inputs."""
    ins = {k: np.asarray(v) for k, v in inputs.items()}
    w_ih0 = ins["w_ih0"].astype(np.float32)
    w_hh0 = ins["w_hh0"].astype(np.float32)
    enc_w = ins["enc_w"].astype(np.float32)
    enc_b = ins["enc_b"].astype(np.float32)
    dec_w = ins["dec_w"].astype(np.float32)
    dec_b = ins["dec_b"].astype(np.float32)
    prop = ins["properties"].astype(np.float32)

    Wx0 = w_ih0[:, :H]
    Wp0 = w_ih0[:, H:]
    A0 = Wx0 @ enc_w                                   # [G, O]
    b0c = Wx0 @ enc_b + ins["b_ih0"] + ins["b_hh0"]    # [G]
    w0aug = np.ascontiguousarray(
        np.concatenate([A0.T, Wp0.T, b0c[None, :].astype(np.float32)], axis=0),
        dtype=np.float32)                              # [52, G]

    def chunked(wT, nk):  # [nk*128, G] -> [128, nk, G]
        return np.ascontiguousarray(
            wT.reshape(nk, 128, wT.shape[1]).transpose(1, 0, 2), dtype=np.float32)

    whh0 = chunked(w_hh0.T, 4)
    W1cat = np.concatenate([ins["w_hh_rest"][0].T, ins["w_ih_rest"][0].T], axis=0)
    W2cat = np.concatenate([ins["w_hh_rest"][1].T, ins["w_ih_rest"][1].T], axis=0)
    w1 = chunked(W1cat.astype(np.float32), 8)
    w2 = chunked(W2cat.astype(np.float32), 8)
    decT_pad = np.zeros((H, OP), np.float32)
    decT_pad[:, :O] = dec_w.T
    dec = chunked(decT_pad, 4)                         # [128, 4, OP]
    b1 = (ins["b_ih_rest"][0] + ins["b_hh_rest"][0]).astype(np.float32)[None, :]
    b2 = (ins["b_ih_rest"][1] + ins["b_hh_rest"][1]).astype(np.float32)[None, :]

    shared = {
        "w0aug": w0aug, "whh0": whh0, "w1": w1, "w2": w2, "dec": dec,
        "b1": np.ascontiguousarray(b1), "b2": np.ascontiguousarray(b2),
        "dec_b": np.ascontiguousarray(
            np.concatenate([dec_b, np.zeros(OP - O, np.float32)])[None, :], dtype=np.float32),
        "init": _init_const(),
    }
    in_maps = []
    for cid in range(NCORES):
        xaug = np.zeros((KAUG, BL), np.float32)
        xaug[1, :] = 1.0                               # logits_init = onehot(1)
        xaug[O:O + P, :] = prop[cid * BL:(cid + 1) * BL, :].T
        xaug[O + P, :] = 1.0
        in_maps.append({**shared, "xaug0": np.ascontiguousarray(xaug)})
    return in_maps


_NC_CACHE = {}


def _run(inputs, t_steps):
    from concourse.bass_utils import run_bass_kernel_spmd

    if t_steps not in _NC_CACHE:
        _NC_CACHE[t_steps] = _build_nc(t_steps)
    nc = _NC_CACHE[t_steps]
    in_maps = _host_fold(inputs)
    res = run_bass_kernel_spmd(nc, in_maps, core_ids=list(range(NCORES)))
    outs = [res.results[cid]["out"].reshape(BL, t_steps, O) for cid in range(NCORES)]
    return np.concatenate(outs, axis=0)


def kernel(**inputs):
    t_steps = np.asarray(inputs["x"]).shape[1]
    return _run(inputs, t_steps)
